# revision 6
# baseline (speedup 1.0000x reference)
"""Causal self-attention kernel for Trainium2, sharded over 8 NeuronCores.

Problem: x:(2048,2,768) f32, 12 heads, head_dim 64.
Sharding: batch (2) x head-groups (4 groups of 3 heads) -> 8 cores.
Each core computes q/k/v projections for its (batch, 3 heads), causal
flash-style attention, and a partial c_proj contribution. The host sums the
4 partial outputs per batch (the "all-reduce") and adds bo.

Device-side layout notes:
  - Everything PE-bound runs as fp32r (fp22-truncated fp32, full PE rate at
    moving-dim >= 256).
  - Scores are computed TRANSPOSED: scoresT[t, s] so softmax's denominator
    comes from a ones-column appended to V (m=65 matmul) and the exp runs
    along the free axis; no PE transposes of the probability matrix needed.
  - Causal masking: diagonal 128x512 score tiles restrict the live column
    range (lo) and a gpsimd affine_select zeroes the triangular remainder.
"""

import os
import sys

sys.path.insert(0, "/opt/trn_rl_repo")

import numpy as np

import concourse.bass as bass  # noqa: F401  (import keeps bass registered)
import concourse.tile as tile
from concourse import bacc, bass_utils, library_config, mybir

F32 = mybir.dt.float32
F32R = mybir.dt.float32r
BF16 = mybir.dt.bfloat16

S = 2048          # sequence length
B = 2             # batch
D = 768           # d_model
H = 12            # total heads
HD = 64           # head dim
NH = 3            # heads per core
DKL = NH * HD     # local q/k/v width = 192
KT = 6            # k-tiles over D (6 x 128)
SB = 512          # s-block width
QB = S // SB      # 4 q-blocks
TT = S // 128     # 16 t-tiles
SCALE = 1.0 / np.sqrt(HD)

_PROGRAM_CACHE = {}
LAST_EXEC_NS = None


def _build_program(trace_unused=False):
    nc = bacc.Bacc("TRN2", target_bir_lowering=False, debug=False, num_devices=8)

    xt_d = nc.dram_tensor("xt", [KT, 128, S], F32R, kind="ExternalInput").ap()
    wq_d = nc.dram_tensor("wq", [KT, 128, DKL], F32R, kind="ExternalInput").ap()
    wk_d = nc.dram_tensor("wk", [KT, 128, DKL], F32R, kind="ExternalInput").ap()
    wv_d = nc.dram_tensor("wv", [KT, 128, DKL], F32R, kind="ExternalInput").ap()
    wo_d = nc.dram_tensor("wo", [DKL, D], F32R, kind="ExternalInput").ap()
    ones_d = nc.dram_tensor("ones3", [128, NH, 1], F32R, kind="ExternalInput").ap()
    out_d = nc.dram_tensor("outT", [D, S], BF16, kind="ExternalOutput").ap()

    EXP = mybir.ActivationFunctionType.Exp
    GE = mybir.AluOpType.is_ge
    MUL = mybir.AluOpType.mult

    with tile.TileContext(nc) as tc:
        with (
            tc.tile_pool(name="xp", bufs=1) as xp,
            tc.tile_pool(name="wp", bufs=1) as wp,
            tc.tile_pool(name="qk", bufs=1) as qk,
            tc.tile_pool(name="vp", bufs=1) as vp,
            tc.tile_pool(name="ep", bufs=6) as ep,
            tc.tile_pool(name="ys", bufs=1) as ys,
            tc.tile_pool(name="dn", bufs=4) as dn,
            tc.tile_pool(name="op", bufs=4) as op,
            tc.tile_pool(name="psA", bufs=3, space="PSUM") as psA,
            tc.tile_pool(name="psB", bufs=3, space="PSUM") as psB,
            tc.tile_pool(name="psC", bufs=2, space="PSUM") as psC,
        ):
            nc.gpsimd.load_library(library_config.attn)

            # ---- Phase 0: DMA inputs ----
            xts = []
            for k in range(KT):
                t = xp.tile([128, S], F32R, tag=f"x{k}")
                nc.sync.dma_start(t[:], xt_d[k])
                xts.append(t)
            wqs, wks, wvs = [], [], []
            for name, dram, dst in (("q", wq_d, wqs), ("k", wk_d, wks), ("v", wv_d, wvs)):
                for k in range(KT):
                    t = wp.tile([128, DKL], F32R, tag=f"w{name}{k}")
                    nc.sync.dma_start(t[:], dram[k])
                    dst.append(t)
            wo1 = wp.tile([128, D], F32R, tag="wo1")
            nc.sync.dma_start(wo1[:], wo_d[0:128])
            wo2 = wp.tile([64, D], F32R, tag="wo2")
            nc.sync.dma_start(wo2[:], wo_d[128:DKL])

            qA = qk.tile([128, S], F32R, tag="qA")
            qB_ = qk.tile([64, S], F32R, tag="qB")
            kA = qk.tile([128, S], F32R, tag="kA")
            kB = qk.tile([64, S], F32R, tag="kB")
            yA = ys.tile([128, S], F32R, tag="yA")
            yB = ys.tile([64, S], F32R, tag="yB")

            # ---- Phase 1: qT / kT projections (dk on partitions, s free) ----
            passes = [
                (qA, wqs, 0, 128),
                (kA, wks, 0, 128),
                (qB_, wqs, 128, 64),
                (kB, wks, 128, 64),
            ]
            for dst, ws, m0, mw in passes:
                for ncol in range(QB):
                    ps = psC.tile([128, SB], F32, tag="mm")
                    for k in range(KT):
                        nc.tensor.matmul(
                            ps[0:mw, :],
                            ws[k][:, m0 : m0 + mw],
                            xts[k][:, ncol * SB : (ncol + 1) * SB],
                            start=(k == 0),
                            stop=(k == KT - 1),
                        )
                    nc.vector.tensor_copy(
                        dst[0:mw, ncol * SB : (ncol + 1) * SB], ps[0:mw, :]
                    )

            # ---- Phase 2: V natural layout, ones-augmented: [v0|1|v1|1|v2|1] ----
            vas = []
            for t in range(TT):
                ps = psC.tile([128, SB], F32, tag="mm")
                for k in range(KT):
                    nc.tensor.matmul(
                        ps[:, 0:DKL],
                        xts[k][:, t * 128 : (t + 1) * 128],
                        wvs[k][:],
                        start=(k == 0),
                        stop=(k == KT - 1),
                    )
                va = vp.tile([128, NH * (HD + 1)], F32R, tag=f"v{t}")
                var = va[:].rearrange("p (h c) -> p h c", c=HD + 1)
                nc.sync.dma_start(var[:, :, HD : HD + 1], ones_d)
                nc.vector.tensor_copy(
                    var[:, :, 0:HD],
                    ps[:, 0:DKL].rearrange("p (h d) -> p h d", d=HD),
                )
                vas.append(va)

            # ---- Phase 3: attention, per q-block ----
            for qb in range(QB):
                nt = 4 * qb + 4
                yps = [
                    psB.tile([HD + 1, SB], F32, tag="ya", name=f"yps_{qb}_{h}")
                    for h in range(NH)
                ]
                for t in range(nt):
                    d = t * 128 - qb * SB
                    if d < 0:
                        lo, sw = 0, 0
                    else:
                        lo = min(d, 256)
                        sw = d + 128 - lo
                    w = SB - lo
                    sq = qA[:, qb * SB + lo : (qb + 1) * SB]
                    sqB = qB_[:, qb * SB + lo : (qb + 1) * SB]
                    sps = [
                        psA.tile([128, SB], F32, tag="sc", name=f"sp_{qb}_{t}_{h}")
                        for h in range(NH)
                    ]
                    nc.tensor.matmul(
                        sps[0][:, lo:SB],
                        kA[0:64, t * 128 : (t + 1) * 128],
                        sq[0:64, :],
                        start=True, stop=True,
                    )
                    nc.tensor.matmul(
                        sps[1][:, lo:SB],
                        kA[64:128, t * 128 : (t + 1) * 128],
                        sq[64:128, :],
                        start=True, stop=True,
                    )
                    nc.tensor.matmul(
                        sps[2][:, lo:SB],
                        kB[0:64, t * 128 : (t + 1) * 128],
                        sqB[0:64, :],
                        start=True, stop=True,
                    )
                    for h in range(NH):
                        ex = ep.tile([128, SB], F32R, tag="exp")
                        nc.scalar.activation(
                            ex[:, lo:SB], sps[h][:, lo:SB], EXP, scale=float(SCALE)
                        )
                        if d >= 0:
                            nc.gpsimd.affine_select(
                                out=ex[:, lo : lo + sw],
                                in_=ex[:, lo : lo + sw],
                                compare_op=GE,
                                fill=0.0,
                                base=lo - d,
                                channel_multiplier=-1,
                                pattern=[[1, sw]],
                            )
                        nc.tensor.matmul(
                            yps[h][:, lo:SB],
                            vas[t][:, h * (HD + 1) : (h + 1) * (HD + 1)],
                            ex[:, lo:SB],
                            start=(t == 0),
                            stop=(t == nt - 1),
                        )
                # softmax divide: row 64 of each yps is the denominator
                for h in range(NH):
                    rc = dn.tile([1, SB], F32, tag="rc")
                    nc.vector.reciprocal(rc[:], yps[h][HD : HD + 1, :])
                    bc = dn.tile([64, SB], F32, tag="bc")
                    nc.gpsimd.partition_broadcast(bc[:], rc[:], channels=64)
                    if h == 0:
                        dst = yA[0:64, qb * SB : (qb + 1) * SB]
                    elif h == 1:
                        dst = yA[64:128, qb * SB : (qb + 1) * SB]
                    else:
                        dst = yB[0:64, qb * SB : (qb + 1) * SB]
                    nc.vector.tensor_tensor(dst, yps[h][0:HD, :], bc[:], MUL)

            # ---- Phase 4: c_proj partial: outT = Wo_s.T-contract over dv ----
            for ncol in range(QB):
                for mc in range(D // 128):
                    ps = psC.tile([128, SB], F32, tag="mm")
                    nc.tensor.matmul(
                        ps[:],
                        wo1[:, mc * 128 : (mc + 1) * 128],
                        yA[:, ncol * SB : (ncol + 1) * SB],
                        start=True, stop=False,
                    )
                    nc.tensor.matmul(
                        ps[:],
                        wo2[:, mc * 128 : (mc + 1) * 128],
                        yB[:, ncol * SB : (ncol + 1) * SB],
                        start=False, stop=True,
                    )
                    st = op.tile([128, SB], BF16, tag="st")
                    nc.vector.tensor_copy(st[:], ps[:])
                    nc.sync.dma_start(
                        out_d[mc * 128 : (mc + 1) * 128, ncol * SB : (ncol + 1) * SB],
                        st[:],
                    )

    nc.compile()
    return nc


def kernel(x, Wq, bq, Wk, bk, Wv, bv, Wo, bo):
    global LAST_EXEC_NS
    x = np.asarray(x, dtype=np.float32)
    Wq = np.asarray(Wq, dtype=np.float32)
    Wk = np.asarray(Wk, dtype=np.float32)
    Wv = np.asarray(Wv, dtype=np.float32)
    Wo = np.asarray(Wo, dtype=np.float32)
    bq = np.asarray(bq, dtype=np.float32)
    bk = np.asarray(bk, dtype=np.float32)
    bv = np.asarray(bv, dtype=np.float32)
    bo = np.asarray(bo, dtype=np.float32)

    # The device program folds no biases; fold nonzero bq/bk/bv by augmenting
    # x with a constant column would change shapes, so handle the (unused in
    # this problem) nonzero case on the host by a reference fallback.
    if np.any(bq) or np.any(bk) or np.any(bv):
        q = (x @ Wq + bq).reshape(S, B, H, HD)
        k = (x @ Wk + bk).reshape(S, B, H, HD)
        v = (x @ Wv + bv).reshape(S, B, H, HD)
        att = np.einsum("sbhd,tbhd->bhst", q, k) * SCALE
        causal = np.triu(np.ones((S, S), dtype=bool), k=1)
        att = np.where(causal[None, None], -np.inf, att)
        att = att - att.max(axis=-1, keepdims=True)
        att = np.exp(att)
        att = att / att.sum(axis=-1, keepdims=True)
        y = np.einsum("bhst,tbhd->sbhd", att, v).reshape(S, B, D)
        return (y @ Wo + bo).astype(np.float32)

    if "prog" not in _PROGRAM_CACHE:
        _PROGRAM_CACHE["prog"] = _build_program()
    nc = _PROGRAM_CACHE["prog"]

    in_maps = []
    xT = [np.ascontiguousarray(x[:, b, :].T) for b in range(B)]  # (768, 2048)
    for c in range(8):
        b, g = c // 4, c % 4
        sl = slice(g * DKL, (g + 1) * DKL)
        in_maps.append({
            "xt": xT[b].reshape(KT, 128, S),
            "wq": np.ascontiguousarray(Wq[:, sl]).reshape(KT, 128, DKL),
            "wk": np.ascontiguousarray(Wk[:, sl]).reshape(KT, 128, DKL),
            "wv": np.ascontiguousarray(Wv[:, sl]).reshape(KT, 128, DKL),
            "wo": np.ascontiguousarray(Wo[sl, :]),
            "ones3": np.ones((128, NH, 1), dtype=np.float32),
        })

    trace = bool(int(os.environ.get("KERNEL_TRACE", "0")))
    res = bass_utils.run_bass_kernel_spmd(
        nc, in_maps, core_ids=list(range(8)), trace=trace
    )
    LAST_EXEC_NS = res.exec_time_ns

    out = np.zeros((S, B, D), dtype=np.float32)
    for c in range(8):
        b = c // 4
        out[:, b, :] += res.results[c]["outT"].astype(np.float32).T
    out += bo
    return out


# revision 11
# speedup vs baseline: 1.3768x; 1.3768x over previous
"""Causal self-attention kernel for Trainium2, sharded over 8 NeuronCores.

Problem: x:(2048,2,768) f32, 12 heads, head_dim 64.
Sharding: batch (2) x head-groups (4 groups of 3 heads) -> 8 cores.
Each core computes q/k/v projections for its (batch, 3 heads), causal
flash-style attention, and a partial c_proj contribution. The host sums the
4 partial outputs per batch (the "all-reduce") and adds bo.

Device-side layout notes:
  - Matmul operands are bf16 (1 cycle/row on PE; fp32 accumulate in PSUM).
  - Scores are computed TRANSPOSED: scoresT[t, s] so softmax's denominator
    comes from a ones-column appended to V (m=65 matmul) and the exp runs
    along the free axis; no PE transposes of the probability matrix needed.
  - Causal masking: diagonal 128x512 score tiles restrict the live column
    range (lo) and a gpsimd affine_select zeroes the triangular remainder.
"""

import os
import sys

sys.path.insert(0, "/opt/trn_rl_repo")

import numpy as np

import concourse.bass as bass  # noqa: F401  (import keeps bass registered)
import concourse.tile as tile
from concourse import bacc, bass_utils, library_config, mybir

F32 = mybir.dt.float32
F32R = mybir.dt.float32r
BF16 = mybir.dt.bfloat16

S = 2048          # sequence length
B = 2             # batch
D = 768           # d_model
H = 12            # total heads
HD = 64           # head dim
NH = 3            # heads per core
DKL = NH * HD     # local q/k/v width = 192
KT = 6            # k-tiles over D (6 x 128)
SB = 512          # s-block width
QB = S // SB      # 4 q-blocks
TT = S // 128     # 16 t-tiles
SCALE = 1.0 / np.sqrt(HD)

_PROGRAM_CACHE = {}
LAST_EXEC_NS = None


def _build_program(trace_unused=False):
    nc = bacc.Bacc("TRN2", target_bir_lowering=False, debug=False, num_devices=8)

    xt_d = nc.dram_tensor("xt", [KT, 128, S], BF16, kind="ExternalInput").ap()
    wq_d = nc.dram_tensor("wq", [KT, 128, DKL], BF16, kind="ExternalInput").ap()
    wk_d = nc.dram_tensor("wk", [KT, 128, DKL], BF16, kind="ExternalInput").ap()
    wv_d = nc.dram_tensor("wv", [KT, 128, DKL], BF16, kind="ExternalInput").ap()
    wo_d = nc.dram_tensor("wo", [DKL, D], BF16, kind="ExternalInput").ap()
    ones_d = nc.dram_tensor("ones3", [128, NH, 1], BF16, kind="ExternalInput").ap()
    out_d = nc.dram_tensor("outT", [D, S], BF16, kind="ExternalOutput").ap()

    EXP = mybir.ActivationFunctionType.Exp
    GE = mybir.AluOpType.is_ge
    MUL = mybir.AluOpType.mult

    with tile.TileContext(nc) as tc:
        with (
            tc.tile_pool(name="xp", bufs=1) as xp,
            tc.tile_pool(name="wp", bufs=1) as wp,
            tc.tile_pool(name="qk", bufs=1) as qk,
            tc.tile_pool(name="vp", bufs=1) as vp,
            tc.tile_pool(name="ep", bufs=6) as ep,
            tc.tile_pool(name="ys", bufs=1) as ys,
            tc.tile_pool(name="dn", bufs=4) as dn,
            tc.tile_pool(name="op", bufs=4) as op,
            tc.tile_pool(name="psA", bufs=3, space="PSUM") as psA,
            tc.tile_pool(name="psB", bufs=3, space="PSUM") as psB,
            tc.tile_pool(name="psC", bufs=2, space="PSUM") as psC,
        ):
            nc.gpsimd.load_library(library_config.attn)

            # ---- Phase 0: DMA inputs ----
            xts = []
            for k in range(KT):
                t = xp.tile([128, S], BF16, tag=f"x{k}")
                nc.sync.dma_start(t[:], xt_d[k])
                xts.append(t)
            wqs, wks, wvs = [], [], []
            for name, dram, dst in (("q", wq_d, wqs), ("k", wk_d, wks), ("v", wv_d, wvs)):
                for k in range(KT):
                    t = wp.tile([128, DKL], BF16, tag=f"w{name}{k}")
                    nc.sync.dma_start(t[:], dram[k])
                    dst.append(t)
            wo1 = wp.tile([128, D], BF16, tag="wo1")
            nc.sync.dma_start(wo1[:], wo_d[0:128])
            wo2 = wp.tile([64, D], BF16, tag="wo2")
            nc.sync.dma_start(wo2[:], wo_d[128:DKL])

            qA = qk.tile([128, S], BF16, tag="qA")
            qB_ = qk.tile([64, S], BF16, tag="qB")
            kA = qk.tile([128, S], BF16, tag="kA")
            kB = qk.tile([64, S], BF16, tag="kB")
            yA = ys.tile([128, S], BF16, tag="yA")
            yB = ys.tile([64, S], BF16, tag="yB")

            # ---- Phase 1: qT / kT projections (dk on partitions, s free) ----
            for dst, ws in ((qA, wqs), (kA, wks)):
                for ncol in range(QB):
                    ps = psC.tile([128, SB], F32, tag="mm")
                    for k in range(KT):
                        nc.tensor.matmul(
                            ps[:],
                            ws[k][:, 0:128],
                            xts[k][:, ncol * SB : (ncol + 1) * SB],
                            start=(k == 0),
                            stop=(k == KT - 1),
                        )
                    nc.vector.tensor_copy(
                        dst[:, ncol * SB : (ncol + 1) * SB], ps[:]
                    )
            # tail 64 columns of q and k (m=64 passes)
            for dst, ws in ((qB_, wqs), (kB, wks)):
                for ncol in range(QB):
                    ps = psC.tile([128, SB], F32, tag="mm")
                    for k in range(KT):
                        nc.tensor.matmul(
                            ps[0:64, :],
                            ws[k][:, 128:192],
                            xts[k][:, ncol * SB : (ncol + 1) * SB],
                            start=(k == 0),
                            stop=(k == KT - 1),
                        )
                    nc.vector.tensor_copy(dst[:, ncol * SB : (ncol + 1) * SB], ps[0:64, :])

            # ---- Phase 2: V natural layout, ones-augmented: [v0|1|v1|1|v2|1] ----
            vas = []
            for t in range(TT):
                ps = psC.tile([128, SB], F32, tag="mm")
                for k in range(KT):
                    nc.tensor.matmul(
                        ps[:, 0:DKL],
                        xts[k][:, t * 128 : (t + 1) * 128],
                        wvs[k][:],
                        start=(k == 0),
                        stop=(k == KT - 1),
                    )
                va = vp.tile([128, NH * (HD + 1)], BF16, tag=f"v{t}")
                var = va[:].rearrange("p (h c) -> p h c", c=HD + 1)
                nc.sync.dma_start(var[:, :, HD : HD + 1], ones_d)
                nc.vector.tensor_copy(
                    var[:, :, 0:HD],
                    ps[:, 0:DKL].rearrange("p (h d) -> p h d", d=HD),
                )
                vas.append(va)

            # ---- Phase 3: attention, per q-block ----
            for qb in range(QB):
                nt = 4 * qb + 4
                yps = [
                    psB.tile([HD + 1, SB], F32, tag="ya", name=f"yps_{qb}_{h}")
                    for h in range(NH)
                ]
                for t in range(nt):
                    d = t * 128 - qb * SB
                    if d < 0:
                        lo, sw = 0, 0
                    else:
                        lo = min(d, 256)
                        sw = d + 128 - lo
                    w = SB - lo
                    sq = qA[:, qb * SB + lo : (qb + 1) * SB]
                    sqB = qB_[:, qb * SB + lo : (qb + 1) * SB]
                    sps = [
                        psA.tile([128, SB], F32, tag="sc", name=f"sp_{qb}_{t}_{h}")
                        for h in range(NH)
                    ]
                    nc.tensor.matmul(
                        sps[0][:, lo:SB],
                        kA[0:64, t * 128 : (t + 1) * 128],
                        sq[0:64, :],
                        start=True, stop=True,
                    )
                    nc.tensor.matmul(
                        sps[1][:, lo:SB],
                        kA[64:128, t * 128 : (t + 1) * 128],
                        sq[64:128, :],
                        start=True, stop=True,
                    )
                    nc.tensor.matmul(
                        sps[2][:, lo:SB],
                        kB[0:64, t * 128 : (t + 1) * 128],
                        sqB[0:64, :],
                        start=True, stop=True,
                    )
                    for h in range(NH):
                        ex = ep.tile([128, SB], BF16, tag="exp")
                        nc.scalar.activation(
                            ex[:, lo:SB], sps[h][:, lo:SB], EXP, scale=float(SCALE)
                        )
                        if d >= 0:
                            nc.gpsimd.affine_select(
                                out=ex[:, lo : lo + sw],
                                in_=ex[:, lo : lo + sw],
                                compare_op=GE,
                                fill=0.0,
                                base=lo - d,
                                channel_multiplier=-1,
                                pattern=[[1, sw]],
                            )
                        nc.tensor.matmul(
                            yps[h][:, lo:SB],
                            vas[t][:, h * (HD + 1) : (h + 1) * (HD + 1)],
                            ex[:, lo:SB],
                            start=(t == 0),
                            stop=(t == nt - 1),
                        )
                # softmax divide: row 64 of each yps is the denominator
                for h in range(NH):
                    dr = dn.tile([1, SB], F32, tag="dr")
                    nc.vector.tensor_copy(dr[:], yps[h][HD : HD + 1, :])
                    rc = dn.tile([1, SB], F32, tag="rc")
                    nc.vector.reciprocal_approx_fast(rc[:], dr[:])
                    bc = dn.tile([64, SB], F32, tag="bc")
                    nc.gpsimd.partition_broadcast(bc[:], rc[:], channels=64)
                    if h == 0:
                        dst = yA[0:64, qb * SB : (qb + 1) * SB]
                    elif h == 1:
                        dst = yA[64:128, qb * SB : (qb + 1) * SB]
                    else:
                        dst = yB[0:64, qb * SB : (qb + 1) * SB]
                    nc.vector.tensor_tensor(dst, yps[h][0:HD, :], bc[:], MUL)

            # ---- Phase 4: c_proj partial: outT = Wo_s.T-contract over dv ----
            for ncol in range(QB):
                for mc in range(D // 128):
                    ps = psC.tile([128, SB], F32, tag="mm")
                    nc.tensor.matmul(
                        ps[:],
                        wo1[:, mc * 128 : (mc + 1) * 128],
                        yA[:, ncol * SB : (ncol + 1) * SB],
                        start=True, stop=False,
                    )
                    nc.tensor.matmul(
                        ps[:],
                        wo2[:, mc * 128 : (mc + 1) * 128],
                        yB[:, ncol * SB : (ncol + 1) * SB],
                        start=False, stop=True,
                    )
                    st = op.tile([128, SB], BF16, tag="st")
                    nc.vector.tensor_copy(st[:], ps[:])
                    nc.sync.dma_start(
                        out_d[mc * 128 : (mc + 1) * 128, ncol * SB : (ncol + 1) * SB],
                        st[:],
                    )

    nc.compile()
    return nc


def kernel(x, Wq, bq, Wk, bk, Wv, bv, Wo, bo):
    global LAST_EXEC_NS
    x = np.asarray(x, dtype=np.float32)
    Wq = np.asarray(Wq, dtype=np.float32)
    Wk = np.asarray(Wk, dtype=np.float32)
    Wv = np.asarray(Wv, dtype=np.float32)
    Wo = np.asarray(Wo, dtype=np.float32)
    bq = np.asarray(bq, dtype=np.float32)
    bk = np.asarray(bk, dtype=np.float32)
    bv = np.asarray(bv, dtype=np.float32)
    bo = np.asarray(bo, dtype=np.float32)

    # The device program folds no biases; fold nonzero bq/bk/bv by augmenting
    # x with a constant column would change shapes, so handle the (unused in
    # this problem) nonzero case on the host by a reference fallback.
    if np.any(bq) or np.any(bk) or np.any(bv):
        q = (x @ Wq + bq).reshape(S, B, H, HD)
        k = (x @ Wk + bk).reshape(S, B, H, HD)
        v = (x @ Wv + bv).reshape(S, B, H, HD)
        att = np.einsum("sbhd,tbhd->bhst", q, k) * SCALE
        causal = np.triu(np.ones((S, S), dtype=bool), k=1)
        att = np.where(causal[None, None], -np.inf, att)
        att = att - att.max(axis=-1, keepdims=True)
        att = np.exp(att)
        att = att / att.sum(axis=-1, keepdims=True)
        y = np.einsum("bhst,tbhd->sbhd", att, v).reshape(S, B, D)
        return (y @ Wo + bo).astype(np.float32)

    if "prog" not in _PROGRAM_CACHE:
        _PROGRAM_CACHE["prog"] = _build_program()
    nc = _PROGRAM_CACHE["prog"]

    import ml_dtypes

    bf = ml_dtypes.bfloat16
    in_maps = []
    xT = [np.ascontiguousarray(x[:, b, :].T).astype(bf) for b in range(B)]
    for c in range(8):
        b, g = c // 4, c % 4
        sl = slice(g * DKL, (g + 1) * DKL)
        in_maps.append({
            "xt": xT[b].reshape(KT, 128, S),
            "wq": np.ascontiguousarray(Wq[:, sl]).astype(bf).reshape(KT, 128, DKL),
            "wk": np.ascontiguousarray(Wk[:, sl]).astype(bf).reshape(KT, 128, DKL),
            "wv": np.ascontiguousarray(Wv[:, sl]).astype(bf).reshape(KT, 128, DKL),
            "wo": np.ascontiguousarray(Wo[sl, :]).astype(bf),
            "ones3": np.ones((128, NH, 1), dtype=bf),
        })

    trace = bool(int(os.environ.get("KERNEL_TRACE", "0")))
    res = bass_utils.run_bass_kernel_spmd(
        nc, in_maps, core_ids=list(range(8)), trace=trace
    )
    LAST_EXEC_NS = res.exec_time_ns

    out = np.zeros((S, B, D), dtype=np.float32)
    for c in range(8):
        b = c // 4
        out[:, b, :] += res.results[c]["outT"].astype(np.float32).T
    out += bo
    return out


# revision 13
# speedup vs baseline: 1.5314x; 1.1123x over previous
"""Causal self-attention kernel for Trainium2, sharded over 8 NeuronCores.

Problem: x:(2048,2,768) f32, 12 heads, head_dim 64.
Sharding: batch (2) x head-groups (4 groups of 3 heads) -> 8 cores.
Each core computes q/k/v projections for its (batch, 3 heads), causal
flash-style attention, and a partial c_proj contribution. The host sums the
4 partial outputs per batch (the "all-reduce") and adds bo.

Device-side layout notes:
  - Matmul operands are bf16 (1 cycle/row on PE; fp32 accumulate in PSUM).
  - Scores are computed TRANSPOSED: scoresT[t, s] so softmax's denominator
    comes from a ones-column appended to V (m=65 matmul) and the exp runs
    along the free axis; no PE transposes of the probability matrix needed.
  - Causal masking: diagonal 128x512 score tiles restrict the live column
    range (lo) and a gpsimd affine_select zeroes the triangular remainder.
"""

import os
import sys

sys.path.insert(0, "/opt/trn_rl_repo")

import numpy as np

import concourse.bass as bass  # noqa: F401  (import keeps bass registered)
import concourse.tile as tile
from concourse import bacc, bass_utils, library_config, mybir

F32 = mybir.dt.float32
F32R = mybir.dt.float32r
BF16 = mybir.dt.bfloat16

S = 2048          # sequence length
B = 2             # batch
D = 768           # d_model
H = 12            # total heads
HD = 64           # head dim
NH = 3            # heads per core
DKL = NH * HD     # local q/k/v width = 192
KT = 6            # k-tiles over D (6 x 128)
SB = 512          # s-block width
QB = S // SB      # 4 q-blocks
TT = S // 128     # 16 t-tiles
SCALE = 1.0 / np.sqrt(HD)

_PROGRAM_CACHE = {}
LAST_EXEC_NS = None


def _build_program(trace_unused=False):
    nc = bacc.Bacc("TRN2", target_bir_lowering=False, debug=False, num_devices=8)

    xt_d = nc.dram_tensor("xt", [KT, 128, S], BF16, kind="ExternalInput").ap()
    wq_d = nc.dram_tensor("wq", [KT, 128, DKL], BF16, kind="ExternalInput").ap()
    wk_d = nc.dram_tensor("wk", [KT, 128, DKL], BF16, kind="ExternalInput").ap()
    wv_d = nc.dram_tensor("wv", [KT, 128, DKL], BF16, kind="ExternalInput").ap()
    wo_d = nc.dram_tensor("wo", [DKL, D], BF16, kind="ExternalInput").ap()
    ones_d = nc.dram_tensor("ones3", [128, NH, 1], BF16, kind="ExternalInput").ap()
    out_d = nc.dram_tensor("outT", [D, S], BF16, kind="ExternalOutput").ap()

    EXP = mybir.ActivationFunctionType.Exp
    GE = mybir.AluOpType.is_ge
    MUL = mybir.AluOpType.mult

    with tile.TileContext(nc) as tc:
        with (
            tc.tile_pool(name="xp", bufs=1) as xp,
            tc.tile_pool(name="wp", bufs=1) as wp,
            tc.tile_pool(name="qk", bufs=1) as qk,
            tc.tile_pool(name="vp", bufs=1) as vp,
            tc.tile_pool(name="ep", bufs=6) as ep,
            tc.tile_pool(name="ys", bufs=1) as ys,
            tc.tile_pool(name="dn", bufs=4) as dn,
            tc.tile_pool(name="op", bufs=1) as op,
            tc.tile_pool(name="psA", bufs=3, space="PSUM") as psA,
            tc.tile_pool(name="psB", bufs=3, space="PSUM") as psB,
            tc.tile_pool(name="psC", bufs=2, space="PSUM") as psC,
        ):
            nc.gpsimd.load_library(library_config.attn)

            # ---- Phase 0: DMA inputs (weights first so PE can start early) ----
            wqs, wks, wvs = [], [], []
            for name, dram, dst in (("q", wq_d, wqs), ("k", wk_d, wks), ("v", wv_d, wvs)):
                for k in range(KT):
                    t = wp.tile([128, DKL], BF16, tag=f"w{name}{k}")
                    nc.sync.dma_start(t[:], dram[k])
                    dst.append(t)
            xts = []
            for k in range(KT):
                t = xp.tile([128, S], BF16, tag=f"x{k}")
                nc.sync.dma_start(t[:], xt_d[k])
                xts.append(t)
            wo1 = wp.tile([128, D], BF16, tag="wo1")
            nc.sync.dma_start(wo1[:], wo_d[0:128])
            wo2 = wp.tile([64, D], BF16, tag="wo2")
            nc.sync.dma_start(wo2[:], wo_d[128:DKL])

            qA = qk.tile([128, S], BF16, tag="qA")
            qB_ = qk.tile([64, S], BF16, tag="qB")
            kA = qk.tile([128, S], BF16, tag="kA")
            kB = qk.tile([64, S], BF16, tag="kB")
            yA = ys.tile([128, S], BF16, tag="yA")
            yB = ys.tile([64, S], BF16, tag="yB")
            outst = [
                op.tile([128, S], BF16, tag=f"o{mc}", name=f"outst{mc}")
                for mc in range(D // 128)
            ]

            # ---- Phase 1: qT / kT projections (dk on partitions, s free) ----
            for dst, ws in ((qA, wqs), (kA, wks)):
                for ncol in range(QB):
                    ps = psC.tile([128, SB], F32, tag="mm")
                    for k in range(KT):
                        nc.tensor.matmul(
                            ps[:],
                            ws[k][:, 0:128],
                            xts[k][:, ncol * SB : (ncol + 1) * SB],
                            start=(k == 0),
                            stop=(k == KT - 1),
                        )
                    nc.vector.tensor_copy(
                        dst[:, ncol * SB : (ncol + 1) * SB], ps[:]
                    )
            # tail 64 columns of q and k, col-packed into one PE pass
            for ncol in range(QB):
                psq = psC.tile([128, SB], F32, tag="mm", name=f"psq{ncol}")
                psk = psC.tile([128, SB], F32, tag="mm", name=f"psk{ncol}")
                for k in range(KT):
                    rhs = xts[k][:, ncol * SB : (ncol + 1) * SB]
                    nc.tensor.matmul(
                        psq[0:64, :], wqs[k][:, 128:192], rhs,
                        start=(k == 0), stop=(k == KT - 1),
                        tile_position=(0, 0),
                    )
                    nc.tensor.matmul(
                        psk[64:128, :], wks[k][:, 128:192], rhs,
                        start=(k == 0), stop=(k == KT - 1),
                        tile_position=(0, 64),
                    )
                nc.vector.tensor_copy(qB_[:, ncol * SB : (ncol + 1) * SB], psq[0:64, :])
                nc.vector.tensor_copy(kB[:, ncol * SB : (ncol + 1) * SB], psk[64:128, :])

            # ---- Phase 2: V natural layout, ones-augmented: [v0|1|v1|1|v2|1] ----
            vas = []
            for t in range(TT):
                ps = psC.tile([128, SB], F32, tag="mm")
                for k in range(KT):
                    nc.tensor.matmul(
                        ps[:, 0:DKL],
                        xts[k][:, t * 128 : (t + 1) * 128],
                        wvs[k][:],
                        start=(k == 0),
                        stop=(k == KT - 1),
                    )
                va = vp.tile([128, NH * (HD + 1)], BF16, tag=f"v{t}")
                var = va[:].rearrange("p (h c) -> p h c", c=HD + 1)
                nc.sync.dma_start(var[:, :, HD : HD + 1], ones_d)
                nc.vector.tensor_copy(
                    var[:, :, 0:HD],
                    ps[:, 0:DKL].rearrange("p (h d) -> p h d", d=HD),
                )
                vas.append(va)

            # ---- Phase 3: attention, per q-block ----
            for qb in range(QB):
                nt = 4 * qb + 4
                yps = [
                    psB.tile([HD + 1, SB], F32, tag="ya", name=f"yps_{qb}_{h}")
                    for h in range(NH)
                ]
                for t in range(nt):
                    d = t * 128 - qb * SB
                    if d < 0:
                        lo, sw = 0, 0
                    else:
                        lo, sw = d, 128
                    w = SB - lo
                    sq = qA[:, qb * SB + lo : (qb + 1) * SB]
                    sqB = qB_[:, qb * SB + lo : (qb + 1) * SB]
                    sps = [
                        psA.tile([128, SB], F32, tag="sc", name=f"sp_{qb}_{t}_{h}")
                        for h in range(NH)
                    ]
                    nc.tensor.matmul(
                        sps[0][:, lo:SB],
                        kA[0:64, t * 128 : (t + 1) * 128],
                        sq[0:64, :],
                        start=True, stop=True,
                    )
                    nc.tensor.matmul(
                        sps[1][:, lo:SB],
                        kA[64:128, t * 128 : (t + 1) * 128],
                        sq[64:128, :],
                        start=True, stop=True,
                    )
                    nc.tensor.matmul(
                        sps[2][:, lo:SB],
                        kB[0:64, t * 128 : (t + 1) * 128],
                        sqB[0:64, :],
                        start=True, stop=True,
                    )
                    for h in range(NH):
                        ex = ep.tile([128, SB], BF16, tag="exp")
                        nc.scalar.activation(
                            ex[:, lo:SB], sps[h][:, lo:SB], EXP, scale=float(SCALE)
                        )
                        if d >= 0:
                            nc.gpsimd.affine_select(
                                out=ex[:, lo : lo + sw],
                                in_=ex[:, lo : lo + sw],
                                compare_op=GE,
                                fill=0.0,
                                base=lo - d,
                                channel_multiplier=-1,
                                pattern=[[1, sw]],
                            )
                        nc.tensor.matmul(
                            yps[h][:, lo:SB],
                            vas[t][:, h * (HD + 1) : (h + 1) * (HD + 1)],
                            ex[:, lo:SB],
                            start=(t == 0),
                            stop=(t == nt - 1),
                        )
                # softmax divide: row 64 of each yps is the denominator
                for h in range(NH):
                    dr = dn.tile([1, SB], F32, tag="dr")
                    nc.vector.tensor_copy(dr[:], yps[h][HD : HD + 1, :])
                    rc = dn.tile([1, SB], F32, tag="rc")
                    nc.vector.reciprocal_approx_fast(rc[:], dr[:])
                    bc = dn.tile([64, SB], F32, tag="bc")
                    nc.gpsimd.partition_broadcast(bc[:], rc[:], channels=64)
                    if h == 0:
                        dst = yA[0:64, qb * SB : (qb + 1) * SB]
                    elif h == 1:
                        dst = yA[64:128, qb * SB : (qb + 1) * SB]
                    else:
                        dst = yB[0:64, qb * SB : (qb + 1) * SB]
                    nc.vector.tensor_tensor(dst, yps[h][0:HD, :], bc[:], MUL)

                # c_proj for this s-chunk (overlaps next q-block's attention)
                for mc in range(D // 128):
                    ps = psC.tile([128, SB], F32, tag="mm", name=f"cp_{qb}_{mc}")
                    nc.tensor.matmul(
                        ps[:],
                        wo1[:, mc * 128 : (mc + 1) * 128],
                        yA[:, qb * SB : (qb + 1) * SB],
                        start=True, stop=False,
                    )
                    nc.tensor.matmul(
                        ps[:],
                        wo2[:, mc * 128 : (mc + 1) * 128],
                        yB[:, qb * SB : (qb + 1) * SB],
                        start=False, stop=True,
                    )
                    nc.vector.tensor_copy(
                        outst[mc][:, qb * SB : (qb + 1) * SB], ps[:]
                    )
                    if qb == QB - 1:
                        nc.sync.dma_start(
                            out_d[mc * 128 : (mc + 1) * 128, :], outst[mc][:]
                        )


    nc.compile()
    return nc


def kernel(x, Wq, bq, Wk, bk, Wv, bv, Wo, bo):
    global LAST_EXEC_NS
    x = np.asarray(x, dtype=np.float32)
    Wq = np.asarray(Wq, dtype=np.float32)
    Wk = np.asarray(Wk, dtype=np.float32)
    Wv = np.asarray(Wv, dtype=np.float32)
    Wo = np.asarray(Wo, dtype=np.float32)
    bq = np.asarray(bq, dtype=np.float32)
    bk = np.asarray(bk, dtype=np.float32)
    bv = np.asarray(bv, dtype=np.float32)
    bo = np.asarray(bo, dtype=np.float32)

    # The device program folds no biases; fold nonzero bq/bk/bv by augmenting
    # x with a constant column would change shapes, so handle the (unused in
    # this problem) nonzero case on the host by a reference fallback.
    if np.any(bq) or np.any(bk) or np.any(bv):
        q = (x @ Wq + bq).reshape(S, B, H, HD)
        k = (x @ Wk + bk).reshape(S, B, H, HD)
        v = (x @ Wv + bv).reshape(S, B, H, HD)
        att = np.einsum("sbhd,tbhd->bhst", q, k) * SCALE
        causal = np.triu(np.ones((S, S), dtype=bool), k=1)
        att = np.where(causal[None, None], -np.inf, att)
        att = att - att.max(axis=-1, keepdims=True)
        att = np.exp(att)
        att = att / att.sum(axis=-1, keepdims=True)
        y = np.einsum("bhst,tbhd->sbhd", att, v).reshape(S, B, D)
        return (y @ Wo + bo).astype(np.float32)

    if "prog" not in _PROGRAM_CACHE:
        _PROGRAM_CACHE["prog"] = _build_program()
    nc = _PROGRAM_CACHE["prog"]

    import ml_dtypes

    bf = ml_dtypes.bfloat16
    in_maps = []
    xT = [np.ascontiguousarray(x[:, b, :].T).astype(bf) for b in range(B)]
    for c in range(8):
        b, g = c // 4, c % 4
        sl = slice(g * DKL, (g + 1) * DKL)
        in_maps.append({
            "xt": xT[b].reshape(KT, 128, S),
            "wq": np.ascontiguousarray(Wq[:, sl]).astype(bf).reshape(KT, 128, DKL),
            "wk": np.ascontiguousarray(Wk[:, sl]).astype(bf).reshape(KT, 128, DKL),
            "wv": np.ascontiguousarray(Wv[:, sl]).astype(bf).reshape(KT, 128, DKL),
            "wo": np.ascontiguousarray(Wo[sl, :]).astype(bf),
            "ones3": np.ones((128, NH, 1), dtype=bf),
        })

    trace = bool(int(os.environ.get("KERNEL_TRACE", "0")))
    res = bass_utils.run_bass_kernel_spmd(
        nc, in_maps, core_ids=list(range(8)), trace=trace
    )
    LAST_EXEC_NS = res.exec_time_ns

    out = np.zeros((S, B, D), dtype=np.float32)
    for c in range(8):
        b = c // 4
        out[:, b, :] += res.results[c]["outT"].astype(np.float32).T
    out += bo
    return out
